# revision 10
# baseline (speedup 1.0000x reference)
"""KAN layer Trainium2 kernel.

Math: out[b,o] = sum_{i,g} exp(-|tanh(x[b,i]) - grid[g]| * s[o,i]) * w[o,i,g]

For t = tanh(x) in grid interval v (grid[v] <= t < grid[v+1]):
    f_{o,i}(t) = P_v * e^{-s t} + S_{v+1} * e^{s t}
with P_v = sum_{j<=v} w_j e^{s g_j}, S_{v+1} = sum_{j>v} w_j e^{-s g_j}.
Each piece is expanded in a degree-(NCHEB-1) Chebyshev basis of the
interval-local coordinate d = 7t + 6 - 2v, giving

    out[b,o] = sum_{i,v,c} mask_v(t[b,i]) * T_c(d[b,i]) * D[(v,c,i), o]

i.e. one (B x NV*NCHEB*I) @ (NV*NCHEB*I x O) matmul, 8-way data-parallel
over the batch. D is precomputed on the host (weight-only preprocessing).

Device structure per core (BSH=128 batch rows):
- interval index k = round(3.5 t + 3) via the fp32 +M/-M trick
  (M = 1.5*2^23; every immediate used anywhere is bf16-exact because the
  tensor_scalar immediate path quantizes, e.g. 12582911.5 behaves as M).
- per i-half a packed (128, 4*BSH) bf16 basis tile [1 | d | T2 | T3].
- per (v, i-half) ONE tensor_tensor: (k==v) mask broadcast x NCHEB times
  the packed basis -> the four matmul lhsT slices at once; these TTs
  alternate VectorE / GpSimd (GpSimd supports plain TT, not TS/STT).
- both matmul operands bf16 (fp32 matmuls issue as 2 HI/LO instructions
  with slow weight loads; bf16 enables FWL); 56 accumulating matmuls.
- D arrives as 4 large DMAs (many small DMAs measured only ~171 GB/s).
- dummy matmuls on a zeroed tile while the basis is computed open the
  PE 2.4 GHz clock gate before the real matmul stream.
"""

import numpy as np
import ml_dtypes

B, I, O, G = 1024, 256, 256, 8
NV = G - 1            # 7 intervals
NCHEB = 4             # degree-3 Chebyshev per interval
N_CORES = 8
BSH = B // N_CORES    # 128 batch rows per core
IH = I // 128         # 2 partition halves of the i dimension
NWARM = 10            # dummy matmuls to open the PE clock gate
VHALF = 4             # D streamed in halves v<4 / v>=4

_CACHE = {}


def _precompute_dmat(spline_weight, spline_scaler, grid):
    """D as (IH, 128, NV*NCHEB*O) bf16: [hh, i, ((v*NCHEB + c)*O + o)]."""
    w = spline_weight.astype(np.float64)          # (O, I, G)
    s = spline_scaler.astype(np.float64)          # (O, I)
    g = grid.astype(np.float64)                   # (G,)
    OI = O * I

    Eg = np.exp(g[None, None, :] * s[:, :, None])             # (O,I,G)
    P = np.cumsum(w * Eg, axis=2)                              # prefix_j<=v
    S = np.cumsum((w / Eg)[:, :, ::-1], axis=2)[:, :, ::-1]    # suffix_j>=v

    h = 1.0 / NV
    centers = -1.0 + (2 * np.arange(NV) + 1) * h

    # Chebyshev coefs of e^{-s h d}, d in [-1,1], via node projection
    M = 32
    nodes = np.cos(np.pi * (np.arange(M) + 0.5) / M)
    Tn = np.cos(np.outer(np.arange(NCHEB), np.arccos(nodes)))  # (NCHEB, M)
    proj = Tn.T * (2.0 / M)
    proj[:, 0] *= 0.5
    sf = s.reshape(-1)                                          # (O*I,)
    Fm = np.exp(-np.outer(sf * h, nodes))                       # (O*I, M)
    Am = Fm @ proj                                              # coefs of e^{-s h d}
    Ap = (1.0 / Fm) @ proj                                      # coefs of e^{+s h d}

    Pf = P.reshape(OI, G)
    Sf = S.reshape(OI, G)
    D = np.empty((NV, NCHEB, OI))
    for v in range(NV):
        em = np.exp(-sf * centers[v])
        pc = Pf[:, v] * em
        sc = Sf[:, v + 1] / em
        D[v] = (pc[:, None] * Am + sc[:, None] * Ap).T          # (NCHEB, O*I)
    # (NV, NCHEB, O, I) -> (IH, 128i, NV*NCHEB*O)
    Dm = D.reshape(NV, NCHEB, O, IH, 128).transpose(3, 4, 0, 1, 2)
    Dm = Dm.reshape(IH, 128, NV * NCHEB * O)
    return np.ascontiguousarray(Dm).astype(ml_dtypes.bfloat16)


def _build_module():
    import concourse.bacc as bacc
    import concourse.bass as bass
    import concourse.mybir as mybir
    import concourse.tile as tile

    f32 = mybir.dt.float32
    bf16 = mybir.dt.bfloat16
    AF = mybir.ActivationFunctionType
    ALU = mybir.AluOpType

    nc = bacc.Bacc("TRN2", target_bir_lowering=False, debug=False,
                   num_devices=N_CORES)

    CW0 = VHALF * NCHEB * O                 # columns in D half 0
    CW1 = (NV - VHALF) * NCHEB * O          # columns in D half 1
    xT = nc.dram_tensor("xt", [IH, 128, BSH], f32, kind="ExternalInput")
    dmat = nc.dram_tensor("dmat", [IH, 128, NV * NCHEB * O], bf16,
                          kind="ExternalInput")
    out_d = nc.dram_tensor("out", [BSH, O], f32, kind="ExternalOutput")

    with tile.TileContext(nc) as tc:
        with (
            tc.tile_pool(name="keep", bufs=1) as keep,
            tc.tile_pool(name="dpool", bufs=1) as dpool,
            tc.tile_pool(name="prod", bufs=1) as prod,
            tc.tile_pool(name="psum", bufs=1, space=bass.MemorySpace.PSUM) as ppool,
        ):
            # x tiles first on the DMA queue, then the 4 big D chunks in
            # matmul consumption order.
            xsb = [None] * IH
            for hh in range(IH):
                xsb[hh] = keep.tile([128, BSH], f32, tag=f"x{hh}", name=f"x{hh}")
                nc.sync.dma_start(xsb[hh][:], xT[hh])
            dhalf = [[None, None], [None, None]]   # [hh][half]
            for half, cw in ((0, CW0), (1, CW1)):
                for hh in range(IH):
                    dt_ = dpool.tile([128, cw], bf16, tag=f"d{hh}_{half}",
                                     name=f"d{hh}_{half}")
                    nc.sync.dma_start(
                        dt_[:], dmat[hh, :, half * CW0:half * CW0 + cw])
                    dhalf[hh][half] = dt_

            # Dummy matmuls to open the PE clock gate during basis compute.
            wz = keep.tile([128, 512], bf16, tag="warm", name="warm")
            nc.vector.memset(wz[:], 0.0)
            wps = ppool.tile([128, 512], f32, tag="wps", name="wps")
            for _ in range(NWARM):
                nc.tensor.matmul(wps[:], wz[:, :128], wz[:],
                                 start=True, stop=True)

            # Basis per i-half into tcat = [1 | d | T2 | T3] (bf16).
            # k = round(3.5t+3) (fp32 +M/-M round trick; RTE ties at grid
            # points are harmless), d = (7t+6) - 2k in [-1,1].
            MAGIC = 12582912.0  # 1.5 * 2^23
            kcat = keep.tile([128, IH * BSH], bf16, tag="kcat", name="kcat")
            tcat = [None] * IH
            for hh in range(IH):
                t = keep.tile([128, BSH], f32, tag=f"t{hh}")
                nc.scalar.activation(t[:], xsb[hh][:], AF.Tanh)
                ua = keep.tile([128, BSH], f32, tag=f"ua{hh}")
                nc.vector.tensor_scalar(ua[:], t[:], 3.5, 3.0, ALU.mult, ALU.add)
                r1 = keep.tile([128, BSH], f32, tag=f"r1{hh}")
                nc.vector.tensor_scalar(r1[:], ua[:], MAGIC, None, ALU.add)
                kf = keep.tile([128, BSH], f32, tag=f"kf{hh}")
                nc.vector.tensor_scalar(kf[:], r1[:], MAGIC, None, ALU.subtract)
                nc.vector.tensor_scalar(kcat[:, hh * BSH:(hh + 1) * BSH],
                                        r1[:], MAGIC, None, ALU.subtract)
                u7 = keep.tile([128, BSH], f32, tag=f"u7{hh}")
                nc.vector.tensor_scalar(u7[:], t[:], 7.0, 6.0, ALU.mult, ALU.add)

                tc_ = keep.tile([128, (NCHEB - 1) * BSH], bf16,
                                tag=f"tcat{hh}", name=f"tcat{hh}")
                db = tc_[:, 0:BSH]
                nc.vector.scalar_tensor_tensor(db, kf[:], -2.0, u7[:],
                                               ALU.mult, ALU.add)
                d2 = keep.tile([128, BSH], bf16, tag=f"d2{hh}")
                nc.vector.tensor_tensor(d2[:], db, db, ALU.mult)
                t2 = tc_[:, BSH:2 * BSH]
                nc.vector.tensor_scalar(t2, d2[:], 2.0, -1.0, ALU.mult, ALU.add)
                u3 = keep.tile([128, BSH], bf16, tag=f"u3{hh}")
                nc.vector.tensor_scalar(u3[:], t2, 2.0, -1.0, ALU.mult, ALU.add)
                t3 = tc_[:, 2 * BSH:3 * BSH]
                nc.vector.tensor_tensor(t3, db, u3[:], ALU.mult)
                tcat[hh] = tc_

            # Masks, fused products, matmul stream.
            acc = ppool.tile([BSH, O], f32, tag="acc", name="acc")
            n_mm = NV * NCHEB * IH
            idx = 0
            for v in range(NV):
                mv = keep.tile([128, IH * BSH], bf16, tag="m", name=f"m{v}")
                nc.vector.tensor_scalar(mv[:], kcat[:], float(v), None,
                                        ALU.is_equal)
                half = 0 if v < VHALF else 1
                voff = (v - half * VHALF) * NCHEB * O
                for hh in range(IH):
                    pt = prod.tile([128, (NCHEB - 1) * BSH], bf16,
                                   tag=f"p{v}_{hh}", name=f"p{v}_{hh}")
                    nc.vector.tensor_tensor(
                        pt[:].rearrange("p (c n) -> p c n", c=NCHEB - 1),
                        mv[:, hh * BSH:(hh + 1) * BSH].unsqueeze(1)
                            .to_broadcast((128, NCHEB - 1, BSH)),
                        tcat[hh][:].rearrange("p (c n) -> p c n", c=NCHEB - 1),
                        ALU.mult)
                    for c in range(NCHEB):
                        lt = (mv[:, hh * BSH:(hh + 1) * BSH] if c == 0
                              else pt[:, (c - 1) * BSH:c * BSH])
                        nc.tensor.matmul(
                            acc[:], lt,
                            dhalf[hh][half][:, voff + c * O:voff + (c + 1) * O],
                            start=(idx == 0), stop=(idx == n_mm - 1))
                        idx += 1

            osb = keep.tile([BSH, O], f32, tag="o", name="o")
            nc.scalar.copy(osb[:], acc[:])
            nc.sync.dma_start(out_d[:], osb[:])

    nc.compile()
    return nc


def kernel(x, spline_weight, spline_scaler, grid):
    from concourse import bass_utils

    x = np.asarray(x, dtype=np.float32)
    Dm = _precompute_dmat(np.asarray(spline_weight), np.asarray(spline_scaler),
                          np.asarray(grid))

    if "nc" not in _CACHE:
        _CACHE["nc"] = _build_module()
    nc = _CACHE["nc"]

    in_maps = []
    for cid in range(N_CORES):
        xs = x[cid * BSH:(cid + 1) * BSH]                  # (BSH, I)
        xT = np.ascontiguousarray(xs.T.reshape(IH, 128, BSH), dtype=np.float32)
        in_maps.append({"xt": xT, "dmat": Dm})

    import os
    trace = bool(int(os.environ.get("KAN_TRACE", "0")))
    kw = {}
    if trace:
        tdir = os.environ.get("KAN_TRACE_DIR") or None
        kw = dict(trace=True, tmpdir=tdir)
    res = bass_utils.run_bass_kernel_spmd(nc, in_maps,
                                          core_ids=list(range(N_CORES)), **kw)
    _CACHE["last_result"] = res
    out = np.concatenate([res.results[cid]["out"] for cid in range(N_CORES)], axis=0)
    return out.astype(np.float32)


# revision 11
# speedup vs baseline: 1.0846x; 1.0846x over previous
"""KAN layer Trainium2 kernel.

Math: out[b,o] = sum_{i,g} exp(-|tanh(x[b,i]) - grid[g]| * s[o,i]) * w[o,i,g]

For t = tanh(x) in grid interval v (grid[v] <= t < grid[v+1]):
    f_{o,i}(t) = P_v * e^{-s t} + S_{v+1} * e^{s t}
with P_v = sum_{j<=v} w_j e^{s g_j}, S_{v+1} = sum_{j>v} w_j e^{-s g_j}.
Each piece is expanded in a degree-(NCHEB-1) Chebyshev basis of the
interval-local coordinate d = 7t + 6 - 2v, giving

    out[b,o] = sum_{i,v,c} mask_v(t[b,i]) * T_c(d[b,i]) * D[(v,c,i), o]

i.e. one (B x NV*NCHEB*I) @ (NV*NCHEB*I x O) matmul, 8-way data-parallel
over the batch. D is precomputed on the host (weight-only preprocessing).

Device structure per core (BSH=128 batch rows):
- interval index k = round(3.5 t + 3) via the fp32 +M/-M trick
  (M = 1.5*2^23; every immediate used anywhere is bf16-exact because the
  tensor_scalar immediate path quantizes, e.g. 12582911.5 behaves as M).
- per i-half a packed (128, 4*BSH) bf16 basis tile [1 | d | T2 | T3].
- per (v, i-half) ONE tensor_tensor: (k==v) mask broadcast x NCHEB times
  the packed basis -> the four matmul lhsT slices at once; these TTs
  alternate VectorE / GpSimd (GpSimd supports plain TT, not TS/STT).
- both matmul operands bf16 (fp32 matmuls issue as 2 HI/LO instructions
  with slow weight loads; bf16 enables FWL); 56 accumulating matmuls.
- D arrives as 4 large DMAs (many small DMAs measured only ~171 GB/s).
- dummy matmuls on a zeroed tile while the basis is computed open the
  PE 2.4 GHz clock gate before the real matmul stream.
"""

import numpy as np
import ml_dtypes

B, I, O, G = 1024, 256, 256, 8
NV = G - 1            # 7 intervals
NCHEB = 4             # degree-3 Chebyshev per interval
N_CORES = 8
BSH = B // N_CORES    # 128 batch rows per core
IH = I // 128         # 2 partition halves of the i dimension
NWARM = 10            # dummy matmuls to open the PE clock gate
VHALF = 4             # D streamed in halves v<4 / v>=4

_CACHE = {}


def _precompute_dmat(spline_weight, spline_scaler, grid):
    """D as (IH, 128, NV*NCHEB*O) bf16: [hh, i, ((v*NCHEB + c)*O + o)]."""
    w = spline_weight.astype(np.float64)          # (O, I, G)
    s = spline_scaler.astype(np.float64)          # (O, I)
    g = grid.astype(np.float64)                   # (G,)
    OI = O * I

    Eg = np.exp(g[None, None, :] * s[:, :, None])             # (O,I,G)
    P = np.cumsum(w * Eg, axis=2)                              # prefix_j<=v
    S = np.cumsum((w / Eg)[:, :, ::-1], axis=2)[:, :, ::-1]    # suffix_j>=v

    h = 1.0 / NV
    centers = -1.0 + (2 * np.arange(NV) + 1) * h

    # Chebyshev coefs of e^{-s h d}, d in [-1,1], via node projection
    M = 32
    nodes = np.cos(np.pi * (np.arange(M) + 0.5) / M)
    Tn = np.cos(np.outer(np.arange(NCHEB), np.arccos(nodes)))  # (NCHEB, M)
    proj = Tn.T * (2.0 / M)
    proj[:, 0] *= 0.5
    sf = s.reshape(-1)                                          # (O*I,)
    Fm = np.exp(-np.outer(sf * h, nodes))                       # (O*I, M)
    Am = Fm @ proj                                              # coefs of e^{-s h d}
    Ap = (1.0 / Fm) @ proj                                      # coefs of e^{+s h d}

    Pf = P.reshape(OI, G)
    Sf = S.reshape(OI, G)
    D = np.empty((NV, NCHEB, OI))
    for v in range(NV):
        em = np.exp(-sf * centers[v])
        pc = Pf[:, v] * em
        sc = Sf[:, v + 1] / em
        D[v] = (pc[:, None] * Am + sc[:, None] * Ap).T          # (NCHEB, O*I)
    # (NV, NCHEB, O, I) -> (IH, 128i, NV*NCHEB*O)
    Dm = D.reshape(NV, NCHEB, O, IH, 128).transpose(3, 4, 0, 1, 2)
    Dm = Dm.reshape(IH, 128, NV * NCHEB * O)
    return np.ascontiguousarray(Dm).astype(ml_dtypes.bfloat16)


def _build_module():
    import concourse.bacc as bacc
    import concourse.bass as bass
    import concourse.mybir as mybir
    import concourse.tile as tile

    f32 = mybir.dt.float32
    bf16 = mybir.dt.bfloat16
    AF = mybir.ActivationFunctionType
    ALU = mybir.AluOpType

    nc = bacc.Bacc("TRN2", target_bir_lowering=False, debug=False,
                   num_devices=N_CORES)

    CW0 = VHALF * NCHEB * O                 # columns in D half 0
    CW1 = (NV - VHALF) * NCHEB * O          # columns in D half 1
    xT = nc.dram_tensor("xt", [IH, 128, BSH], f32, kind="ExternalInput")
    dmat = nc.dram_tensor("dmat", [IH, 128, NV * NCHEB * O], bf16,
                          kind="ExternalInput")
    out_d = nc.dram_tensor("out", [BSH, O], f32, kind="ExternalOutput")

    with tile.TileContext(nc) as tc:
        with (
            tc.tile_pool(name="keep", bufs=1) as keep,
            tc.tile_pool(name="dpool", bufs=1) as dpool,
            tc.tile_pool(name="prod", bufs=1) as prod,
            tc.tile_pool(name="psum", bufs=1, space=bass.MemorySpace.PSUM) as ppool,
        ):
            # x tiles first on the DMA queue, then the 4 big D chunks in
            # matmul consumption order.
            xsb = [None] * IH
            dhalf = [[None, None], [None, None]]   # [hh][half]
            def _dma_d(hh, half, cw):
                dt_ = dpool.tile([128, cw], bf16, tag=f"d{hh}_{half}",
                                 name=f"d{hh}_{half}")
                nc.sync.dma_start(
                    dt_[:], dmat[hh, :, half * CW0:half * CW0 + cw])
                dhalf[hh][half] = dt_
            _dma_d(0, 0, CW0)
            for hh in range(IH):
                xsb[hh] = keep.tile([128, BSH], f32, tag=f"x{hh}", name=f"x{hh}")
                nc.sync.dma_start(xsb[hh][:], xT[hh])
            _dma_d(1, 0, CW0)
            _dma_d(0, 1, CW1)
            _dma_d(1, 1, CW1)

            # Dummy matmuls to open the PE clock gate during basis compute.
            wz = keep.tile([128, 512], bf16, tag="warm", name="warm")
            nc.vector.memset(wz[:], 0.0)
            wps = ppool.tile([128, 512], f32, tag="wps", name="wps")
            for _ in range(NWARM):
                nc.tensor.matmul(wps[:], wz[:, :128], wz[:],
                                 start=True, stop=True)

            # Basis per i-half into tcat = [1 | d | T2 | T3] (bf16).
            # k = round(3.5t+3) (fp32 +M/-M round trick; RTE ties at grid
            # points are harmless), d = (7t+6) - 2k in [-1,1].
            MAGIC = 12582912.0  # 1.5 * 2^23
            kcat = keep.tile([128, IH * BSH], bf16, tag="kcat", name="kcat")
            tcat = [None] * IH
            for hh in range(IH):
                t = keep.tile([128, BSH], f32, tag=f"t{hh}")
                nc.scalar.activation(t[:], xsb[hh][:], AF.Tanh)
                ua = keep.tile([128, BSH], f32, tag=f"ua{hh}")
                nc.vector.tensor_scalar(ua[:], t[:], 3.5, 3.0, ALU.mult, ALU.add)
                r1 = keep.tile([128, BSH], f32, tag=f"r1{hh}")
                nc.vector.tensor_scalar(r1[:], ua[:], MAGIC, None, ALU.add)
                kf = keep.tile([128, BSH], f32, tag=f"kf{hh}")
                nc.vector.tensor_scalar(kf[:], r1[:], MAGIC, None, ALU.subtract)
                nc.vector.tensor_scalar(kcat[:, hh * BSH:(hh + 1) * BSH],
                                        r1[:], MAGIC, None, ALU.subtract)
                u7 = keep.tile([128, BSH], f32, tag=f"u7{hh}")
                nc.vector.tensor_scalar(u7[:], t[:], 7.0, 6.0, ALU.mult, ALU.add)

                tc_ = keep.tile([128, (NCHEB - 1) * BSH], bf16,
                                tag=f"tcat{hh}", name=f"tcat{hh}")
                db = tc_[:, 0:BSH]
                nc.vector.scalar_tensor_tensor(db, kf[:], -2.0, u7[:],
                                               ALU.mult, ALU.add)
                d2 = keep.tile([128, BSH], bf16, tag=f"d2{hh}")
                nc.vector.tensor_tensor(d2[:], db, db, ALU.mult)
                t2 = tc_[:, BSH:2 * BSH]
                nc.vector.tensor_scalar(t2, d2[:], 2.0, -1.0, ALU.mult, ALU.add)
                u3 = keep.tile([128, BSH], bf16, tag=f"u3{hh}")
                nc.vector.tensor_scalar(u3[:], t2, 2.0, -1.0, ALU.mult, ALU.add)
                t3 = tc_[:, 2 * BSH:3 * BSH]
                nc.vector.tensor_tensor(t3, db, u3[:], ALU.mult)
                tcat[hh] = tc_

            # Masks, fused products, matmul stream.
            acc = ppool.tile([BSH, O], f32, tag="acc", name="acc")
            n_mm = NV * NCHEB * IH
            idx = 0
            for v in range(NV):
                mv = keep.tile([128, IH * BSH], bf16, tag=f"m{v}", name=f"m{v}")
                nc.vector.tensor_scalar(mv[:], kcat[:], float(v), None,
                                        ALU.is_equal)
                half = 0 if v < VHALF else 1
                voff = (v - half * VHALF) * NCHEB * O
                for hh in range(IH):
                    pt = prod.tile([128, (NCHEB - 1) * BSH], bf16,
                                   tag=f"p{v}_{hh}", name=f"p{v}_{hh}")
                    nc.vector.tensor_tensor(
                        pt[:].rearrange("p (c n) -> p c n", c=NCHEB - 1),
                        mv[:, hh * BSH:(hh + 1) * BSH].unsqueeze(1)
                            .to_broadcast((128, NCHEB - 1, BSH)),
                        tcat[hh][:].rearrange("p (c n) -> p c n", c=NCHEB - 1),
                        ALU.mult)
                    for c in range(NCHEB):
                        lt = (mv[:, hh * BSH:(hh + 1) * BSH] if c == 0
                              else pt[:, (c - 1) * BSH:c * BSH])
                        nc.tensor.matmul(
                            acc[:], lt,
                            dhalf[hh][half][:, voff + c * O:voff + (c + 1) * O],
                            start=(idx == 0), stop=(idx == n_mm - 1))
                        idx += 1

            osb = keep.tile([BSH, O], f32, tag="o", name="o")
            nc.scalar.copy(osb[:], acc[:])
            nc.sync.dma_start(out_d[:], osb[:])

    nc.compile()
    return nc


def kernel(x, spline_weight, spline_scaler, grid):
    from concourse import bass_utils

    x = np.asarray(x, dtype=np.float32)
    Dm = _precompute_dmat(np.asarray(spline_weight), np.asarray(spline_scaler),
                          np.asarray(grid))

    if "nc" not in _CACHE:
        _CACHE["nc"] = _build_module()
    nc = _CACHE["nc"]

    in_maps = []
    for cid in range(N_CORES):
        xs = x[cid * BSH:(cid + 1) * BSH]                  # (BSH, I)
        xT = np.ascontiguousarray(xs.T.reshape(IH, 128, BSH), dtype=np.float32)
        in_maps.append({"xt": xT, "dmat": Dm})

    import os
    trace = bool(int(os.environ.get("KAN_TRACE", "0")))
    kw = {}
    if trace:
        tdir = os.environ.get("KAN_TRACE_DIR") or None
        kw = dict(trace=True, tmpdir=tdir)
    res = bass_utils.run_bass_kernel_spmd(nc, in_maps,
                                          core_ids=list(range(N_CORES)), **kw)
    _CACHE["last_result"] = res
    out = np.concatenate([res.results[cid]["out"] for cid in range(N_CORES)], axis=0)
    return out.astype(np.float32)


# revision 12
# speedup vs baseline: 1.1700x; 1.0788x over previous
"""KAN layer Trainium2 kernel.

Math: out[b,o] = sum_{i,g} exp(-|tanh(x[b,i]) - grid[g]| * s[o,i]) * w[o,i,g]

For t = tanh(x) in grid interval v (grid[v] <= t < grid[v+1]):
    f_{o,i}(t) = P_v * e^{-s t} + S_{v+1} * e^{s t}
with P_v = sum_{j<=v} w_j e^{s g_j}, S_{v+1} = sum_{j>v} w_j e^{-s g_j}.
Each piece is expanded in a degree-(NCHEB-1) Chebyshev basis of the
interval-local coordinate d = 7t + 6 - 2v, giving

    out[b,o] = sum_{i,v,c} mask_v(t[b,i]) * T_c(d[b,i]) * D[(v,c,i), o]

i.e. one (B x NV*NCHEB*I) @ (NV*NCHEB*I x O) matmul, 8-way data-parallel
over the batch. D is precomputed on the host (weight-only preprocessing).

Device structure per core (BSH=128 batch rows):
- interval index k = round(3.5 t + 3) via the fp32 +M/-M trick
  (M = 1.5*2^23; every immediate used anywhere is bf16-exact because the
  tensor_scalar immediate path quantizes, e.g. 12582911.5 behaves as M).
- per i-half a packed (128, 4*BSH) bf16 basis tile [1 | d | T2 | T3].
- per (v, i-half) ONE tensor_tensor: (k==v) mask broadcast x NCHEB times
  the packed basis -> the four matmul lhsT slices at once; these TTs
  alternate VectorE / GpSimd (GpSimd supports plain TT, not TS/STT).
- both matmul operands bf16 (fp32 matmuls issue as 2 HI/LO instructions
  with slow weight loads; bf16 enables FWL); 56 accumulating matmuls.
- D arrives as 4 large DMAs (many small DMAs measured only ~171 GB/s).
- dummy matmuls on a zeroed tile while the basis is computed open the
  PE 2.4 GHz clock gate before the real matmul stream.
"""

import numpy as np
import ml_dtypes

B, I, O, G = 1024, 256, 256, 8
NV = G - 1            # 7 intervals
NCHEB = 3             # degree-2 Chebyshev per interval
N_CORES = 8
BSH = B // N_CORES    # 128 batch rows per core
IH = I // 128         # 2 partition halves of the i dimension
NWARM = 10            # dummy matmuls to open the PE clock gate
VCHUNKS = ((0, 2), (2, 4), (4, 6), (6, 7))  # D chunk v-ranges

_CACHE = {}


def _precompute_dmat(spline_weight, spline_scaler, grid):
    """D as (IH, 128, NV*NCHEB*O) bf16: [hh, i, ((v*NCHEB + c)*O + o)]."""
    w = spline_weight.astype(np.float64)          # (O, I, G)
    s = spline_scaler.astype(np.float64)          # (O, I)
    g = grid.astype(np.float64)                   # (G,)
    OI = O * I

    Eg = np.exp(g[None, None, :] * s[:, :, None])             # (O,I,G)
    P = np.cumsum(w * Eg, axis=2)                              # prefix_j<=v
    S = np.cumsum((w / Eg)[:, :, ::-1], axis=2)[:, :, ::-1]    # suffix_j>=v

    h = 1.0 / NV
    centers = -1.0 + (2 * np.arange(NV) + 1) * h

    # Chebyshev coefs of e^{-s h d}, d in [-1,1], via node projection
    M = 32
    nodes = np.cos(np.pi * (np.arange(M) + 0.5) / M)
    Tn = np.cos(np.outer(np.arange(NCHEB), np.arccos(nodes)))  # (NCHEB, M)
    proj = Tn.T * (2.0 / M)
    proj[:, 0] *= 0.5
    sf = s.reshape(-1)                                          # (O*I,)
    Fm = np.exp(-np.outer(sf * h, nodes))                       # (O*I, M)
    Am = Fm @ proj                                              # coefs of e^{-s h d}
    Ap = (1.0 / Fm) @ proj                                      # coefs of e^{+s h d}

    Pf = P.reshape(OI, G)
    Sf = S.reshape(OI, G)
    D = np.empty((NV, NCHEB, OI))
    for v in range(NV):
        em = np.exp(-sf * centers[v])
        pc = Pf[:, v] * em
        sc = Sf[:, v + 1] / em
        D[v] = (pc[:, None] * Am + sc[:, None] * Ap).T          # (NCHEB, O*I)
    # (NV, NCHEB, O, I) -> (IH, 128i, NV*NCHEB*O)
    Dm = D.reshape(NV, NCHEB, O, IH, 128).transpose(3, 4, 0, 1, 2)
    Dm = Dm.reshape(IH, 128, NV * NCHEB * O)
    return np.ascontiguousarray(Dm).astype(ml_dtypes.bfloat16)


def _build_module():
    import concourse.bacc as bacc
    import concourse.bass as bass
    import concourse.mybir as mybir
    import concourse.tile as tile

    f32 = mybir.dt.float32
    bf16 = mybir.dt.bfloat16
    AF = mybir.ActivationFunctionType
    ALU = mybir.AluOpType

    nc = bacc.Bacc("TRN2", target_bir_lowering=False, debug=False,
                   num_devices=N_CORES)

    xT = nc.dram_tensor("xt", [IH, 128, BSH], f32, kind="ExternalInput")
    dmat = nc.dram_tensor("dmat", [IH, 128, NV * NCHEB * O], bf16,
                          kind="ExternalInput")
    out_d = nc.dram_tensor("out", [BSH, O], f32, kind="ExternalOutput")

    with tile.TileContext(nc) as tc:
        with (
            tc.tile_pool(name="keep", bufs=1) as keep,
            tc.tile_pool(name="dpool", bufs=1) as dpool,
            tc.tile_pool(name="prod", bufs=1) as prod,
            tc.tile_pool(name="psum", bufs=1, space=bass.MemorySpace.PSUM) as ppool,
        ):
            # x tiles first on the DMA queue, then the 4 big D chunks in
            # matmul consumption order.
            xsb = [None] * IH
            for hh in range(IH):
                xsb[hh] = keep.tile([128, BSH], f32, tag=f"x{hh}", name=f"x{hh}")
                nc.scalar.dma_start(xsb[hh][:], xT[hh])
            dchunk = [[None] * len(VCHUNKS) for _ in range(IH)]
            for q, (v0, v1) in enumerate(VCHUNKS):
                cw = (v1 - v0) * NCHEB * O
                c0 = v0 * NCHEB * O
                for hh in range(IH):
                    dt_ = dpool.tile([128, cw], bf16, tag=f"d{hh}_{q}",
                                     name=f"d{hh}_{q}")
                    nc.sync.dma_start(dt_[:], dmat[hh, :, c0:c0 + cw])
                    dchunk[hh][q] = dt_

            # Dummy matmuls to open the PE clock gate during basis compute.
            wz = keep.tile([128, 512], bf16, tag="warm", name="warm")
            nc.vector.memset(wz[:], 0.0)
            wps = ppool.tile([128, 512], f32, tag="wps", name="wps")
            for _ in range(NWARM):
                nc.tensor.matmul(wps[:], wz[:, :128], wz[:],
                                 start=True, stop=True)

            # Basis per i-half into tcat = [1 | d | T2 | T3] (bf16).
            # k = round(3.5t+3) (fp32 +M/-M round trick; RTE ties at grid
            # points are harmless), d = (7t+6) - 2k in [-1,1].
            MAGIC = 12582912.0  # 1.5 * 2^23
            kcat = keep.tile([128, IH * BSH], bf16, tag="kcat", name="kcat")
            tcat = [None] * IH
            for hh in range(IH):
                t = keep.tile([128, BSH], f32, tag=f"t{hh}")
                nc.scalar.activation(t[:], xsb[hh][:], AF.Tanh)
                ua = keep.tile([128, BSH], f32, tag=f"ua{hh}")
                nc.vector.tensor_scalar(ua[:], t[:], 3.5, 3.0, ALU.mult, ALU.add)
                r1 = keep.tile([128, BSH], f32, tag=f"r1{hh}")
                nc.vector.tensor_scalar(r1[:], ua[:], MAGIC, None, ALU.add)
                kf = keep.tile([128, BSH], f32, tag=f"kf{hh}")
                nc.vector.tensor_scalar(kf[:], r1[:], MAGIC, None, ALU.subtract)
                nc.vector.tensor_scalar(kcat[:, hh * BSH:(hh + 1) * BSH],
                                        r1[:], MAGIC, None, ALU.subtract)
                u7 = keep.tile([128, BSH], f32, tag=f"u7{hh}")
                nc.vector.tensor_scalar(u7[:], t[:], 7.0, 6.0, ALU.mult, ALU.add)

                tc_ = keep.tile([128, (NCHEB - 1) * BSH], bf16,
                                tag=f"tcat{hh}", name=f"tcat{hh}")
                db = tc_[:, 0:BSH]
                nc.vector.scalar_tensor_tensor(db, kf[:], -2.0, u7[:],
                                               ALU.mult, ALU.add)
                d2 = keep.tile([128, BSH], bf16, tag=f"d2{hh}")
                nc.vector.tensor_tensor(d2[:], db, db, ALU.mult)
                t2 = tc_[:, BSH:2 * BSH]
                nc.vector.tensor_scalar(t2, d2[:], 2.0, -1.0, ALU.mult, ALU.add)
                if NCHEB >= 4:
                    u3 = keep.tile([128, BSH], bf16, tag=f"u3{hh}")
                    nc.vector.tensor_scalar(u3[:], t2, 2.0, -1.0, ALU.mult, ALU.add)
                    t3 = tc_[:, 2 * BSH:3 * BSH]
                    nc.vector.tensor_tensor(t3, db, u3[:], ALU.mult)
                tcat[hh] = tc_

            # Masks, fused products, matmul stream.
            acc = ppool.tile([BSH, O], f32, tag="acc", name="acc")
            n_mm = NV * NCHEB * IH
            idx = 0
            for v in range(NV):
                mv = keep.tile([128, IH * BSH], bf16, tag=f"m{v}", name=f"m{v}")
                nc.vector.tensor_scalar(mv[:], kcat[:], float(v), None,
                                        ALU.is_equal)
                q = next(i for i, (v0, v1) in enumerate(VCHUNKS) if v0 <= v < v1)
                voff = (v - VCHUNKS[q][0]) * NCHEB * O
                for hh in range(IH):
                    pt = prod.tile([128, (NCHEB - 1) * BSH], bf16,
                                   tag=f"p{v}_{hh}", name=f"p{v}_{hh}")
                    nc.vector.tensor_tensor(
                        pt[:].rearrange("p (c n) -> p c n", c=NCHEB - 1),
                        mv[:, hh * BSH:(hh + 1) * BSH].unsqueeze(1)
                            .to_broadcast((128, NCHEB - 1, BSH)),
                        tcat[hh][:].rearrange("p (c n) -> p c n", c=NCHEB - 1),
                        ALU.mult)
                    for c in range(NCHEB):
                        lt = (mv[:, hh * BSH:(hh + 1) * BSH] if c == 0
                              else pt[:, (c - 1) * BSH:c * BSH])
                        nc.tensor.matmul(
                            acc[:], lt,
                            dchunk[hh][q][:, voff + c * O:voff + (c + 1) * O],
                            start=(idx == 0), stop=(idx == n_mm - 1))
                        idx += 1

            osb = keep.tile([BSH, O], f32, tag="o", name="o")
            nc.scalar.copy(osb[:], acc[:])
            nc.scalar.dma_start(out_d[:], osb[:])

    nc.compile()
    return nc


def kernel(x, spline_weight, spline_scaler, grid):
    from concourse import bass_utils

    x = np.asarray(x, dtype=np.float32)
    Dm = _precompute_dmat(np.asarray(spline_weight), np.asarray(spline_scaler),
                          np.asarray(grid))

    if "nc" not in _CACHE:
        _CACHE["nc"] = _build_module()
    nc = _CACHE["nc"]

    in_maps = []
    for cid in range(N_CORES):
        xs = x[cid * BSH:(cid + 1) * BSH]                  # (BSH, I)
        xT = np.ascontiguousarray(xs.T.reshape(IH, 128, BSH), dtype=np.float32)
        in_maps.append({"xt": xT, "dmat": Dm})

    import os
    trace = bool(int(os.environ.get("KAN_TRACE", "0")))
    kw = {}
    if trace:
        tdir = os.environ.get("KAN_TRACE_DIR") or None
        kw = dict(trace=True, tmpdir=tdir)
    res = bass_utils.run_bass_kernel_spmd(nc, in_maps,
                                          core_ids=list(range(N_CORES)), **kw)
    _CACHE["last_result"] = res
    out = np.concatenate([res.results[cid]["out"] for cid in range(N_CORES)], axis=0)
    return out.astype(np.float32)


# revision 13
# speedup vs baseline: 1.2351x; 1.0556x over previous
"""KAN layer Trainium2 kernel.

Math: out[b,o] = sum_{i,g} exp(-|tanh(x[b,i]) - grid[g]| * s[o,i]) * w[o,i,g]

For t = tanh(x) in grid interval v (grid[v] <= t < grid[v+1]):
    f_{o,i}(t) = P_v * e^{-s t} + S_{v+1} * e^{s t}
with P_v = sum_{j<=v} w_j e^{s g_j}, S_{v+1} = sum_{j>v} w_j e^{-s g_j}.
Each piece is expanded in a degree-(NCHEB-1) Chebyshev basis of the
interval-local coordinate d = 7t + 6 - 2v, giving

    out[b,o] = sum_{i,v,c} mask_v(t[b,i]) * T_c(d[b,i]) * D[(v,c,i), o]

i.e. one (B x NV*NCHEB*I) @ (NV*NCHEB*I x O) matmul, 8-way data-parallel
over the batch. D is precomputed on the host (weight-only preprocessing).

Device structure per core (BSH=128 batch rows):
- interval index k = round(3.5 t + 3) via the fp32 +M/-M trick
  (M = 1.5*2^23; every immediate used anywhere is bf16-exact because the
  tensor_scalar immediate path quantizes, e.g. 12582911.5 behaves as M).
- per i-half a packed (128, 4*BSH) bf16 basis tile [1 | d | T2 | T3].
- per (v, i-half) ONE tensor_tensor: (k==v) mask broadcast x NCHEB times
  the packed basis -> the four matmul lhsT slices at once; these TTs
  alternate VectorE / GpSimd (GpSimd supports plain TT, not TS/STT).
- both matmul operands bf16 (fp32 matmuls issue as 2 HI/LO instructions
  with slow weight loads; bf16 enables FWL); 56 accumulating matmuls.
- D arrives as 4 large DMAs (many small DMAs measured only ~171 GB/s).
- dummy matmuls on a zeroed tile while the basis is computed open the
  PE 2.4 GHz clock gate before the real matmul stream.
"""

import numpy as np
import ml_dtypes

B, I, O, G = 1024, 256, 256, 8
NV = G - 1            # 7 intervals
NCHEB = 3             # degree-2 Chebyshev per interval
N_CORES = 8
BSH = B // N_CORES    # 128 batch rows per core
IH = I // 128         # 2 partition halves of the i dimension
NWARM = 10            # dummy matmuls to open the PE clock gate
VCHUNKS = ((0, 2), (2, 4), (4, 6), (6, 7))  # D chunk v-ranges

_CACHE = {}


def _precompute_dmat(spline_weight, spline_scaler, grid):
    """D as (IH, 128, NV*NCHEB*O) bf16: [hh, i, ((v*NCHEB + c)*O + o)]."""
    w = spline_weight.astype(np.float64)          # (O, I, G)
    s = spline_scaler.astype(np.float64)          # (O, I)
    g = grid.astype(np.float64)                   # (G,)
    OI = O * I

    Eg = np.exp(g[None, None, :] * s[:, :, None])             # (O,I,G)
    P = np.cumsum(w * Eg, axis=2)                              # prefix_j<=v
    S = np.cumsum((w / Eg)[:, :, ::-1], axis=2)[:, :, ::-1]    # suffix_j>=v

    h = 1.0 / NV
    centers = -1.0 + (2 * np.arange(NV) + 1) * h

    # Chebyshev coefs of e^{-s h d}, d in [-1,1], via node projection
    M = 32
    nodes = np.cos(np.pi * (np.arange(M) + 0.5) / M)
    Tn = np.cos(np.outer(np.arange(NCHEB), np.arccos(nodes)))  # (NCHEB, M)
    proj = Tn.T * (2.0 / M)
    proj[:, 0] *= 0.5
    sf = s.reshape(-1)                                          # (O*I,)
    Fm = np.exp(-np.outer(sf * h, nodes))                       # (O*I, M)
    Am = Fm @ proj                                              # coefs of e^{-s h d}
    Ap = (1.0 / Fm) @ proj                                      # coefs of e^{+s h d}

    Pf = P.reshape(OI, G)
    Sf = S.reshape(OI, G)
    D = np.empty((NV, NCHEB, OI))
    for v in range(NV):
        em = np.exp(-sf * centers[v])
        pc = Pf[:, v] * em
        sc = Sf[:, v + 1] / em
        D[v] = (pc[:, None] * Am + sc[:, None] * Ap).T          # (NCHEB, O*I)
    # (NV, NCHEB, O, I) -> (IH, 128i, NV*NCHEB*O)
    Dm = D.reshape(NV, NCHEB, O, IH, 128).transpose(3, 4, 0, 1, 2)
    Dm = Dm.reshape(IH, 128, NV * NCHEB * O)
    return np.ascontiguousarray(Dm).astype(ml_dtypes.bfloat16)


def _build_module():
    import concourse.bacc as bacc
    import concourse.bass as bass
    import concourse.mybir as mybir
    import concourse.tile as tile

    f32 = mybir.dt.float32
    bf16 = mybir.dt.bfloat16
    AF = mybir.ActivationFunctionType
    ALU = mybir.AluOpType

    nc = bacc.Bacc("TRN2", target_bir_lowering=False, debug=False,
                   num_devices=N_CORES)

    xT = nc.dram_tensor("xt", [IH, 128, BSH], f32, kind="ExternalInput")
    dmat = nc.dram_tensor("dmat", [IH, 128, NV * NCHEB * O], bf16,
                          kind="ExternalInput")
    out_d = nc.dram_tensor("out", [BSH, O], f32, kind="ExternalOutput")

    with tile.TileContext(nc) as tc:
        with (
            tc.tile_pool(name="keep", bufs=1) as keep,
            tc.tile_pool(name="dpool", bufs=1) as dpool,
            tc.tile_pool(name="prod", bufs=1) as prod,
            tc.tile_pool(name="psum", bufs=1, space=bass.MemorySpace.PSUM) as ppool,
        ):
            # x tiles first on the DMA queue, then the 4 big D chunks in
            # matmul consumption order.
            xsb = [None] * IH
            for hh in range(IH):
                xsb[hh] = keep.tile([128, BSH], f32, tag=f"x{hh}", name=f"x{hh}")
                nc.sync.dma_start(xsb[hh][:], xT[hh])
            dchunk = [[None] * len(VCHUNKS) for _ in range(IH)]
            for q, (v0, v1) in enumerate(VCHUNKS):
                cw = (v1 - v0) * NCHEB * O
                c0 = v0 * NCHEB * O
                for hh in range(IH):
                    dt_ = dpool.tile([128, cw], bf16, tag=f"d{hh}_{q}",
                                     name=f"d{hh}_{q}")
                    nc.sync.dma_start(dt_[:], dmat[hh, :, c0:c0 + cw])
                    dchunk[hh][q] = dt_

            # Dummy matmuls to open the PE clock gate during basis compute.
            wz = keep.tile([128, 512], bf16, tag="warm", name="warm")
            nc.vector.memset(wz[:], 0.0)
            wps = ppool.tile([128, 512], f32, tag="wps", name="wps")
            for _ in range(NWARM):
                nc.tensor.matmul(wps[:], wz[:, :128], wz[:],
                                 start=True, stop=True)

            # Basis per i-half into tcat = [1 | d | T2 | T3] (bf16).
            # k = round(3.5t+3) (fp32 +M/-M round trick; RTE ties at grid
            # points are harmless), d = (7t+6) - 2k in [-1,1].
            MAGIC = 12582912.0  # 1.5 * 2^23
            kcat = keep.tile([128, IH * BSH], bf16, tag="kcat", name="kcat")
            tcat = [None] * IH
            for hh in range(IH):
                t = keep.tile([128, BSH], f32, tag=f"t{hh}")
                nc.scalar.activation(t[:], xsb[hh][:], AF.Tanh)
                ua = keep.tile([128, BSH], f32, tag=f"ua{hh}")
                nc.vector.tensor_scalar(ua[:], t[:], 3.5, 3.0, ALU.mult, ALU.add)
                r1 = keep.tile([128, BSH], f32, tag=f"r1{hh}")
                nc.vector.tensor_scalar(r1[:], ua[:], MAGIC, None, ALU.add)
                kf = keep.tile([128, BSH], f32, tag=f"kf{hh}")
                nc.vector.tensor_scalar(kf[:], r1[:], MAGIC, None, ALU.subtract)
                nc.vector.tensor_scalar(kcat[:, hh * BSH:(hh + 1) * BSH],
                                        r1[:], MAGIC, None, ALU.subtract)
                u7 = keep.tile([128, BSH], f32, tag=f"u7{hh}")
                nc.vector.tensor_scalar(u7[:], t[:], 7.0, 6.0, ALU.mult, ALU.add)

                tc_ = keep.tile([128, (NCHEB - 1) * BSH], bf16,
                                tag=f"tcat{hh}", name=f"tcat{hh}")
                db = tc_[:, 0:BSH]
                nc.vector.scalar_tensor_tensor(db, kf[:], -2.0, u7[:],
                                               ALU.mult, ALU.add)
                d2 = keep.tile([128, BSH], bf16, tag=f"d2{hh}")
                nc.vector.tensor_tensor(d2[:], db, db, ALU.mult)
                t2 = tc_[:, BSH:2 * BSH]
                nc.vector.tensor_scalar(t2, d2[:], 2.0, -1.0, ALU.mult, ALU.add)
                if NCHEB >= 4:
                    u3 = keep.tile([128, BSH], bf16, tag=f"u3{hh}")
                    nc.vector.tensor_scalar(u3[:], t2, 2.0, -1.0, ALU.mult, ALU.add)
                    t3 = tc_[:, 2 * BSH:3 * BSH]
                    nc.vector.tensor_tensor(t3, db, u3[:], ALU.mult)
                tcat[hh] = tc_

            # Masks, fused products, matmul stream.
            acc = ppool.tile([BSH, O], f32, tag="acc", name="acc")
            n_mm = NV * NCHEB * IH
            idx = 0
            for v in range(NV):
                mv = keep.tile([128, IH * BSH], bf16, tag=f"m{v}", name=f"m{v}")
                nc.vector.tensor_scalar(mv[:], kcat[:], float(v), None,
                                        ALU.is_equal)
                q = next(i for i, (v0, v1) in enumerate(VCHUNKS) if v0 <= v < v1)
                voff = (v - VCHUNKS[q][0]) * NCHEB * O
                for hh in range(IH):
                    pt = prod.tile([128, (NCHEB - 1) * BSH], bf16,
                                   tag=f"p{v}_{hh}", name=f"p{v}_{hh}")
                    nc.vector.tensor_tensor(
                        pt[:].rearrange("p (c n) -> p c n", c=NCHEB - 1),
                        mv[:, hh * BSH:(hh + 1) * BSH].unsqueeze(1)
                            .to_broadcast((128, NCHEB - 1, BSH)),
                        tcat[hh][:].rearrange("p (c n) -> p c n", c=NCHEB - 1),
                        ALU.mult)
                    for c in range(NCHEB):
                        lt = (mv[:, hh * BSH:(hh + 1) * BSH] if c == 0
                              else pt[:, (c - 1) * BSH:c * BSH])
                        nc.tensor.matmul(
                            acc[:], lt,
                            dchunk[hh][q][:, voff + c * O:voff + (c + 1) * O],
                            start=(idx == 0), stop=(idx == n_mm - 1))
                        idx += 1

            osb = keep.tile([BSH, O], f32, tag="o", name="o")
            nc.scalar.copy(osb[:], acc[:])
            nc.sync.dma_start(out_d[:], osb[:])

    nc.compile()
    return nc


def kernel(x, spline_weight, spline_scaler, grid):
    from concourse import bass_utils

    x = np.asarray(x, dtype=np.float32)
    Dm = _precompute_dmat(np.asarray(spline_weight), np.asarray(spline_scaler),
                          np.asarray(grid))

    if "nc" not in _CACHE:
        _CACHE["nc"] = _build_module()
    nc = _CACHE["nc"]

    in_maps = []
    for cid in range(N_CORES):
        xs = x[cid * BSH:(cid + 1) * BSH]                  # (BSH, I)
        xT = np.ascontiguousarray(xs.T.reshape(IH, 128, BSH), dtype=np.float32)
        in_maps.append({"xt": xT, "dmat": Dm})

    import os
    trace = bool(int(os.environ.get("KAN_TRACE", "0")))
    kw = {}
    if trace:
        tdir = os.environ.get("KAN_TRACE_DIR") or None
        kw = dict(trace=True, tmpdir=tdir)
    res = bass_utils.run_bass_kernel_spmd(nc, in_maps,
                                          core_ids=list(range(N_CORES)), **kw)
    _CACHE["last_result"] = res
    out = np.concatenate([res.results[cid]["out"] for cid in range(N_CORES)], axis=0)
    return out.astype(np.float32)
